# revision 16
# baseline (speedup 1.0000x reference)
"""DiSAN forward kernel for 8 Trainium2 NeuronCores.

Data-parallel over batch: each of the 8 cores processes B/8 = 2 batch rows.
Per batch b (layout: d-half on partitions for the attention core):
  e    = gather(emb, x[b])                       # indirect DMA [L,D] f32
  h    = elu(e @ Wh + Wh_b)                      # PE bf16 + ACT/DVE elu
  h1b  = (h @ W1 + b)/c ; h2c = (h @ W2)/c       # PE bf16 -> fp16 [l,d]
  per query-group g of 16, per d-half hf:
    zin  = h1b[l,d] + h2c[m,d] via 2 identity-AP matmuls / 4-query chunk
           (PE, PSUM f32)                        # no Pool/DVE broadcast add
    tT   = tanh(zin)    (ACT, PSUM -> fp16)
    E    = exp(c*tT)    (ACT, -> ehe[:,G:2G] bf16)
    EK   = E * keep[m]  (DVE in-place; exact 0 at padded keys)
    EH   = EK * h^T     (DVE, -> ehe[:,0:G])
    off-block sums: direct tensor_reduce XY over ehe block ranges (DVE)
    diag block: dg5 = ehe[diag] * strict-tri masks (Pool), reduce X (DVE)
  fixup: rows with den < 1e-3 (fully masked) get uniform mean_m h[m,d].
  f    = sigmoid(Wf1^T s + Wf2^T h^T + b)        # PE bf16
  u    = f*h + (1-f)*s                           # DVE bf16
  g    = elu(Ws1^T u + b); as = Ws^T g + b       # PE bf16
  out[b] = sum_l u * as                          # DVE accum

Mask-independent program (masks are runtime float inputs): one NEFF, SPMD.
"""

import functools
import numpy as np

import concourse.bass as bass
import concourse.mybir as mybir
from concourse import bacc, tile, masks
from concourse.bass_utils import run_bass_kernel_spmd

P = 128          # partitions / sequence length L
L = 128
D = 256          # model dim
D2 = 512         # 2*D
B = 16           # full batch
NCORES = 8
BLOC = B // NCORES  # batches per core
V = 32000
G = 16           # query group size
NB = L // G      # key blocks (16 wide)
QC = 4           # queries per PSUM-bank matmul chunk
F32 = mybir.dt.float32
BF16 = mybir.dt.bfloat16
FP16 = mybir.dt.float16
AF = mybir.ActivationFunctionType
OP = mybir.AluOpType
AX = mybir.AxisListType


def build_nc(c_val: float, reps: int = 1):
    nc = bacc.Bacc("TRN2", target_bir_lowering=False)

    x_d = nc.dram_tensor("x_idx", [BLOC, P], mybir.dt.int32, kind="ExternalInput")
    emb_d = nc.dram_tensor("emb", [V, D], F32, kind="ExternalInput")
    whw_d = nc.dram_tensor("wh_w", [D, D], F32, kind="ExternalInput")
    whb_d = nc.dram_tensor("wh_b", [1, D], F32, kind="ExternalInput")
    w1w_d = nc.dram_tensor("w1_w", [D, D], F32, kind="ExternalInput")
    w2w_d = nc.dram_tensor("w2_w", [D, D], F32, kind="ExternalInput")
    batt_d = nc.dram_tensor("b_att", [1, D], F32, kind="ExternalInput")
    wf1_d = nc.dram_tensor("wf1_w", [D, D], F32, kind="ExternalInput")
    wf2_d = nc.dram_tensor("wf2_w", [D, D], F32, kind="ExternalInput")
    wf2b_d = nc.dram_tensor("wf2_b", [1, D], F32, kind="ExternalInput")
    ws1_d = nc.dram_tensor("ws1_w", [D2, D2], F32, kind="ExternalInput")
    ws1b_d = nc.dram_tensor("ws1_b", [1, D2], F32, kind="ExternalInput")
    ws_d = nc.dram_tensor("ws_w", [D2, D2], F32, kind="ExternalInput")
    wsb_d = nc.dram_tensor("ws_b", [1, D2], F32, kind="ExternalInput")
    kv_d = nc.dram_tensor("kv", [BLOC, P], F32, kind="ExternalInput")  # 1=keep 0=pad
    out_d = nc.dram_tensor("out", [BLOC, D2], F32, kind="ExternalOutput")

    ic = 1.0 / c_val

    with tile.TileContext(nc) as tc:
        with (
            tc.tile_pool(name="wpool", bufs=1) as wp,
            tc.tile_pool(name="lpool", bufs=1) as lp,
            tc.tile_pool(name="bpool", bufs=2) as bp,
            tc.tile_pool(name="epool", bufs=3) as ep,
            tc.tile_pool(name="scratch", bufs=2) as sp,
            tc.tile_pool(name="psum", bufs=1, space="PSUM") as pp,
            tc.tile_pool(name="psum_z", bufs=1, space="PSUM") as pz,
        ):
            # ---- constants / weights ----
            identf = wp.tile([P, P], F32)
            masks.make_identity(nc, identf[:])
            idb = wp.tile([P, P], BF16)
            nc.vector.tensor_copy(idb[:], identf[:])
            idh = wp.tile([P, P], FP16)
            nc.vector.tensor_copy(idh[:], identf[:])
            ones1 = wp.tile([1, P], F32)
            nc.gpsimd.memset(ones1[:], 1.0)
            ones1b = wp.tile([1, P], BF16)
            nc.gpsimd.memset(ones1b[:], 1.0)
            halfc = wp.tile([P, 1], F32)
            nc.gpsimd.memset(halfc[:], 0.5)

            # strict triangular diag-block masks [dir, (t,q), m]; dir0=fw
            # (m>l); the same triangular pattern is stored for both t slots.
            mask2 = wp.tile([P, 2, 2 * G, G], BF16)
            nc.gpsimd.memset(mask2[:], 1.0)
            for tq in range(2):
                nc.gpsimd.affine_select(
                    out=mask2[:, 0, tq * G : (tq + 1) * G, :], in_=mask2[:, 0, tq * G : (tq + 1) * G, :],
                    compare_op=OP.is_gt, fill=0.0, base=0, channel_multiplier=0,
                    pattern=[[-1, G], [1, G]],  # j - i > 0
                )
                nc.gpsimd.affine_select(
                    out=mask2[:, 1, tq * G : (tq + 1) * G, :], in_=mask2[:, 1, tq * G : (tq + 1) * G, :],
                    compare_op=OP.is_gt, fill=0.0, base=0, channel_multiplier=0,
                    pattern=[[1, G], [-1, G]],  # i - j > 0
                )

            def load_w_bf(dram, kc, n):  # [kc*128, n] -> sbuf bf16 [128, kc, n]
                tf = lp.tile([P, kc, n], F32, tag="wload")
                nc.sync.dma_start(tf[:], dram.rearrange("(c p) n -> p c n", p=P))
                t = wp.tile([P, kc, n], BF16, tag="wb_" + dram.name)
                nc.vector.tensor_copy(t[:], tf[:])
                return t

            whw = load_w_bf(whw_d, 2, D)
            w1w = load_w_bf(w1w_d, 2, D)
            w2w = load_w_bf(w2w_d, 2, D)
            wf1 = load_w_bf(wf1_d, 2, D)
            wf2 = load_w_bf(wf2_d, 2, D)
            ws1 = load_w_bf(ws1_d, 4, D2)
            wsw = load_w_bf(ws_d, 4, D2)

            def load_row_bf(dram, n):
                tf = lp.tile([1, n], F32, tag="rload")
                nc.sync.dma_start(tf[:], dram[:])
                t = wp.tile([1, n], BF16, tag="rb_" + dram.name)
                nc.vector.tensor_copy(t[:], tf[:])
                return t

            whb = load_row_bf(whb_d, D)
            battb = load_row_bf(batt_d, D)
            wf2b = load_row_bf(wf2b_d, D)
            ws1b = load_row_bf(ws1b_d, D2)
            wsb = load_row_bf(wsb_d, D2)

            def prologue(bi):
                st = {}
                # ---- embedding gather ----
                xidx = bp.tile([P, 1], mybir.dt.int32, tag="xidx")
                nc.sync.dma_start(xidx[:], x_d[bi : bi + 1, :].rearrange("o p -> p o"))
                e_sb = bp.tile([P, D], F32, tag="e_sb")
                nc.gpsimd.indirect_dma_start(
                    out=e_sb[:],
                    out_offset=None,
                    in_=emb_d[:],
                    in_offset=bass.IndirectOffsetOnAxis(ap=xidx[:, :1], axis=0),
                )
                e_bf = bp.tile([P, D], BF16, tag="e_bf")
                nc.vector.tensor_copy(e_bf[:], e_sb[:])

                kvrow = bp.tile([1, P], F32, tag="kvrow")
                nc.sync.dma_start(kvrow[:], kv_d[bi : bi + 1, :])

                # QK[d, l] = keep[l] f32 (padded-query zeroing), K128b bf16
                # [d, nb, m] = keep[m] (padded-key mult mask)
                pqk = pp.tile([P, P], F32, tag="t128")
                nc.tensor.matmul(pqk[:], ones1[:], kvrow[:], start=True, stop=True)
                QK = bp.tile([P, P], F32, tag="QK")
                nc.vector.tensor_copy(QK[:], pqk[:])
                K128b = bp.tile([P, NB, G], BF16, tag="K128b")
                nc.vector.tensor_copy(K128b[:], pqk[:])

                # ---- eT (bf16) ----
                eTb = bp.tile([P, 2, P], BF16, tag="eTb")
                for hf in range(2):
                    pt = pp.tile([P, P], BF16, tag="t128b")
                    nc.tensor.matmul(pt[:], e_bf[:, hf * P : (hf + 1) * P], idb[:], is_transpose=True)
                    nc.vector.tensor_copy(eTb[:, hf, :], pt[:])

                # ---- h = elu(e @ Wh + whb), bf16 ----
                ph = pp.tile([P, D], F32, tag="t256")
                nc.tensor.matmul(ph[:], eTb[:, 0, :], whw[:, 0, :], start=True, stop=False)
                nc.tensor.matmul(ph[:], eTb[:, 1, :], whw[:, 1, :], start=False, stop=False)
                nc.tensor.matmul(ph[:], ones1b[:], whb[:], start=False, stop=True)
                hb = bp.tile([P, D], BF16, tag="hb")
                r_ = sp.tile([P, D], BF16, tag="elur")
                m0 = sp.tile([P, D], F32, tag="elum")
                nc.scalar.activation(r_[:], ph[:], AF.Relu)
                nc.vector.tensor_scalar_min(m0[:], ph[:], 0.0)
                nc.scalar.activation(m0[:], m0[:], AF.Exp)
                nc.vector.scalar_tensor_tensor(hb[:], m0[:], 1.0, r_[:], OP.subtract, OP.add)

                # ---- hT (bf16) + sumh ----
                hTb = bp.tile([P, 2, P], BF16, tag="hTb")
                for hf in range(2):
                    pt = pp.tile([P, P], BF16, tag="t128b")
                    nc.tensor.matmul(pt[:], hb[:, hf * P : (hf + 1) * P], idb[:], is_transpose=True)
                    nc.vector.tensor_copy(hTb[:, hf, :], pt[:])
                sumh = bp.tile([P, 2], F32, tag="sumh")
                nc.vector.tensor_reduce(sumh[:], hTb[:], AX.X, OP.add)

                # ---- h1b = (h @ W1 + b)/c ; h2c = (h @ W2)/c, fp16 [l, d] ----
                p1 = pp.tile([P, D], F32, tag="t256")
                nc.tensor.matmul(p1[:], hTb[:, 0, :], w1w[:, 0, :], start=True, stop=False)
                nc.tensor.matmul(p1[:], hTb[:, 1, :], w1w[:, 1, :], start=False, stop=False)
                nc.tensor.matmul(p1[:], ones1b[:], battb[:], start=False, stop=True)
                h1b = bp.tile([P, D], FP16, tag="h1b")
                nc.vector.tensor_scalar_mul(h1b[:], p1[:], ic)
                p2 = pp.tile([P, D], F32, tag="t256")
                nc.tensor.matmul(p2[:], hTb[:, 0, :], w2w[:, 0, :], start=True, stop=False)
                nc.tensor.matmul(p2[:], hTb[:, 1, :], w2w[:, 1, :], start=False, stop=True)
                h2c = bp.tile([P, D], FP16, tag="h2c")
                nc.vector.tensor_scalar_mul(h2c[:], p2[:], ic)

                # khk[t, nb, m]: t=0 -> keep*hT (num), t=1 -> keep (den), per hf
                khk = bp.tile([P, 2, 2, NB, G], BF16, tag="khk")  # [hf, t, nb, m]
                for hf in range(2):
                    nc.vector.tensor_tensor(
                        khk[:, hf, 0, :, :], K128b[:], hTb[:, hf, :], OP.mult
                    )
                    nc.vector.tensor_copy(khk[:, hf, 1, :, :], K128b[:])

                st.update(QK=QK, khk=khk, hTb=hTb, sumh=sumh, h1b=h1b, h2c=h2c)
                return st

            def attention(st):
                khk = st["khk"]
                h1b, h2c = st["h1b"], st["h2c"]
                snsd = bp.tile([P, 2, 2, 2, L], F32, tag="snsd")  # [hf, dir, t, l]
                for hf in range(2):
                    h1s = h1b[:, hf * P : (hf + 1) * P]
                    h2s = h2c[:, hf * P : (hf + 1) * P]
                    for gb in range(NB):
                        g0 = gb * G
                        # zin[d, q, m] = h1b[g0+q, d] + h2c[m, d] on PE
                        zp = pz.tile([P, G, P], F32, tag="zp")
                        for cc in range(G // QC):
                            q0 = g0 + cc * QC
                            nc.tensor.matmul(
                                zp[:, cc * QC : (cc + 1) * QC, :], h1s,
                                idh[:, q0 : q0 + QC].unsqueeze(2).to_broadcast([P, QC, P]),
                                start=True, stop=False,
                            )
                            nc.tensor.matmul(
                                zp[:, cc * QC : (cc + 1) * QC, :], h2s,
                                idh[:].unsqueeze(1).to_broadcast([P, QC, P]),
                                start=False, stop=True,
                            )
                        tT = ep.tile([P, G, P], FP16, tag="tT")
                        nc.scalar.activation(tT[:], zp[:], AF.Tanh)
                        # ehe rows 0:16 = E*K*hT, rows 16:32 = E*K  [(t,q), blk, m]
                        # raw E -> rows 16:32; one fused pass writes
                        # rows 0:16 = E*K*hT (num, t=0) then in-place
                        # rows 16:32 = E*K (den, t=1)
                        ehe = ep.tile([P, 2, G, NB, G], BF16, tag="ehe")
                        nc.scalar.activation(ehe[:, 1, :, :, :], tT[:], AF.Exp, scale=c_val)
                        nc.vector.tensor_tensor(
                            ehe[:],
                            ehe[:, 1, :, :, :].unsqueeze(1).to_broadcast([P, 2, G, NB, G]),
                            khk[:, hf, :, :, :].unsqueeze(2).to_broadcast([P, 2, G, NB, G]),
                            OP.mult,
                        )
                        # off-block sums per direction [dir, t, q] (bf16 out
                        # keeps the DVE reduce in 2x mode)
                        off = ep.tile([P, 2, 2, G], BF16, tag="off")
                        if gb == 0 or gb == NB - 1:
                            nc.gpsimd.memset(off[:], 0.0)
                        with nc.allow_low_precision(reason="bf16 sums; rel tol 2e-2"):
                            if gb < NB - 1:  # fw: blocks after gb
                                nc.vector.tensor_reduce(
                                    off[:, 0, :, :], ehe[:, :, :, gb + 1 : NB, :], AX.XY, OP.add
                                )
                            if gb > 0:       # bw: blocks before gb
                                nc.vector.tensor_reduce(
                                    off[:, 1, :, :], ehe[:, :, :, 0:gb, :], AX.XY, OP.add
                                )
                        # diagonal 16x16 block with strict triangular masks
                        dg5 = ep.tile([P, 2, 2 * G, G], BF16, tag="dg5")
                        nc.gpsimd.tensor_tensor(
                            dg5[:],
                            ehe[:, :, :, gb, :].rearrange("p a b c -> p (a b) c").unsqueeze(1).to_broadcast([P, 2, 2 * G, G]),
                            mask2[:],
                            OP.mult,
                        )
                        dgr = ep.tile([P, 2, 2 * G], BF16, tag="dgr")
                        with nc.allow_low_precision(reason="bf16 sums; rel tol 2e-2"):
                            nc.vector.tensor_reduce(dgr[:], dg5[:], AX.X, OP.add)
                        nc.vector.tensor_tensor(
                            snsd[:, hf, :, :, g0 : g0 + G], off[:], dgr[:], OP.add
                        )
                return snsd

            def downstream(bi, st, snsd):
                QK, hTb, sumh = st["QK"], st["hTb"], st["sumh"]
                # ---- s = num/den, den<thresh -> uniform mean_m h ----
                Sb = bp.tile([P, 2, 2, L], BF16, tag="Sb")  # [hf, dir, l]
                for hf in range(2):
                    # padded queries -> 0 so the uniform fixup fires for them
                    nc.vector.tensor_tensor(
                        snsd[:, hf, :, :, :], snsd[:, hf, :, :, :],
                        QK[:].unsqueeze(1).unsqueeze(1).to_broadcast([P, 2, 2, L]),
                        OP.mult,
                    )
                    sn = snsd[:, hf, :, 0, :]  # [P, 2dir, L]
                    sd = snsd[:, hf, :, 1, :]
                    flag = sp.tile([P, 2, L], F32, tag="flag")
                    nc.vector.tensor_scalar(flag[:], sd, 1e-3, None, OP.is_lt)
                    sd2 = sp.tile([P, 2, L], F32, tag="sd2")
                    nc.vector.scalar_tensor_tensor(sd2[:], flag[:], float(L), sd, OP.mult, OP.add)
                    rd = sp.tile([P, 2, L], F32, tag="rd")
                    nc.vector.reciprocal(rd[:], sd2[:])
                    sn2 = sp.tile([P, 2, L], F32, tag="sn2")
                    nc.vector.scalar_tensor_tensor(
                        sn2[:], flag[:], sumh[:, hf : hf + 1], sn, OP.mult, OP.add
                    )
                    nc.vector.tensor_tensor(Sb[:, hf, :, :], sn2[:], rd[:], OP.mult)

                # ---- fT, uT per direction ----
                UT = bp.tile([P, 4, P], BF16, tag="UT")  # k-chunks: fw0 fw1 bw0 bw1
                for dr in range(2):
                    fT = sp.tile([P, 2, P], BF16, tag="fT")
                    for hf in range(2):
                        pf = pp.tile([P, P], F32, tag="t128")
                        nc.tensor.matmul(pf[:], wf1[:, 0, hf * P : (hf + 1) * P], Sb[:, 0, dr, :], start=True, stop=False)
                        nc.tensor.matmul(pf[:], wf1[:, 1, hf * P : (hf + 1) * P], Sb[:, 1, dr, :], start=False, stop=False)
                        nc.tensor.matmul(pf[:], wf2[:, 0, hf * P : (hf + 1) * P], hTb[:, 0, :], start=False, stop=False)
                        nc.tensor.matmul(pf[:], wf2[:, 1, hf * P : (hf + 1) * P], hTb[:, 1, :], start=False, stop=False)
                        nc.tensor.matmul(pf[:], wf2b[:, hf * P : (hf + 1) * P], ones1b[:], start=False, stop=True)
                        # sigmoid(x) = 0.5*tanh(0.5*x) + 0.5 stays in the
                        # exp/tanh ACT table set (no table switch)
                        th = sp.tile([P, P], BF16, tag="sigth")
                        nc.scalar.activation(th[:], pf[:], AF.Tanh, scale=0.5)
                        nc.scalar.activation(fT[:, hf, :], th[:], AF.Identity, bias=halfc[:], scale=0.5)
                    for hf in range(2):
                        d1 = sp.tile([P, P], BF16, tag="u1")
                        nc.vector.tensor_sub(d1[:], hTb[:, hf, :], Sb[:, hf, dr, :])
                        nc.vector.tensor_mul(d1[:], fT[:, hf, :], d1[:])
                        nc.vector.tensor_add(UT[:, dr * 2 + hf, :], d1[:], Sb[:, hf, dr, :])

                # ---- gT = elu(Ws1^T u^T + ws1b) ----
                gT = bp.tile([P, 4, P], BF16, tag="gT")
                for jc in range(4):
                    pg = pp.tile([P, P], F32, tag="t128")
                    for kc in range(4):
                        nc.tensor.matmul(
                            pg[:], ws1[:, kc, jc * P : (jc + 1) * P], UT[:, kc, :],
                            start=(kc == 0), stop=False,
                        )
                    nc.tensor.matmul(pg[:], ws1b[:, jc * P : (jc + 1) * P], ones1b[:], start=False, stop=True)
                    rg = sp.tile([P, P], BF16, tag="elur2")
                    mg = sp.tile([P, P], F32, tag="elum2")
                    nc.scalar.activation(rg[:], pg[:], AF.Relu)
                    nc.vector.tensor_scalar_min(mg[:], pg[:], 0.0)
                    nc.scalar.activation(mg[:], mg[:], AF.Exp)
                    nc.vector.scalar_tensor_tensor(gT[:, jc, :], mg[:], 1.0, rg[:], OP.subtract, OP.add)

                # ---- att_s and final reduction ----
                outc = bp.tile([P, 4], F32, tag="outc")
                for jc in range(4):
                    pa = pp.tile([P, P], F32, tag="t128")
                    for kc in range(4):
                        nc.tensor.matmul(
                            pa[:], wsw[:, kc, jc * P : (jc + 1) * P], gT[:, kc, :],
                            start=(kc == 0), stop=False,
                        )
                    nc.tensor.matmul(pa[:], wsb[:, jc * P : (jc + 1) * P], ones1b[:], start=False, stop=True)
                    scr = sp.tile([P, P], F32, tag="fin")
                    nc.vector.scalar_tensor_tensor(
                        scr[:], UT[:, jc, :], 1.0, pa[:],
                        OP.mult, OP.mult,
                        accum_out=outc[:, jc : jc + 1],
                    )

                nc.sync.dma_start(
                    out_d[bi : bi + 1, :].rearrange("o (c p) -> p (o c)", p=P), outc[:]
                )

            # software pipeline: prologue of step i+1 issues before the
            # attention+downstream of step i so all engines stay fed across
            # batch boundaries.
            steps = [bi for _ in range(reps) for bi in range(BLOC)]
            cur = prologue(steps[0])
            for i, bi in enumerate(steps):
                nxt = prologue(steps[i + 1]) if i + 1 < len(steps) else None
                snsd = attention(cur)
                downstream(bi, cur, snsd)
                cur = nxt

    nc.compile()
    return nc


@functools.lru_cache(maxsize=6)
def _cached_nc(c_val: float, reps: int = 1):
    return build_nc(c_val, reps)


def build_in_maps(inputs):
    x = np.asarray(inputs["x"])
    mask = np.asarray(inputs["mask"])
    f32 = lambda a: np.ascontiguousarray(np.asarray(a), dtype=np.float32)
    common = {
        "emb": f32(inputs["emb"]),
        "wh_w": f32(inputs["Wh_w"]), "wh_b": f32(inputs["Wh_b"]).reshape(1, D),
        "w1_w": f32(inputs["W1_w"]), "w2_w": f32(inputs["W2_w"]),
        "b_att": f32(inputs["b"]).reshape(1, D),
        "wf1_w": f32(inputs["Wf1_w"]), "wf2_w": f32(inputs["Wf2_w"]),
        "wf2_b": f32(inputs["Wf2_b"]).reshape(1, D),
        "ws1_w": f32(inputs["Ws1_w"]), "ws1_b": f32(inputs["Ws1_b"]).reshape(1, D2),
        "ws_w": f32(inputs["Ws_w"]), "ws_b": f32(inputs["Ws_b"]).reshape(1, D2),
    }
    kv_full = (~mask).astype(np.float32)  # 1.0 = keep, 0.0 = pad
    in_maps = []
    for ci in range(NCORES):
        sl = slice(ci * BLOC, (ci + 1) * BLOC)
        in_maps.append({
            **common,
            "x_idx": np.ascontiguousarray(x[sl].astype(np.int32)),
            "kv": np.ascontiguousarray(kv_full[sl]),
        })
    return in_maps


def kernel(x, mask, emb, Wh_w, Wh_b, W1_w, W2_w, b, c, Wf1_w, Wf2_w, Wf2_b,
           Ws1_w, Ws1_b, Ws_w, Ws_b):
    c_val = float(np.asarray(c).reshape(-1)[0])
    nc = _cached_nc(c_val)
    in_maps = build_in_maps({
        "x": x, "mask": mask, "emb": emb, "Wh_w": Wh_w, "Wh_b": Wh_b,
        "W1_w": W1_w, "W2_w": W2_w, "b": b, "Wf1_w": Wf1_w, "Wf2_w": Wf2_w,
        "Wf2_b": Wf2_b, "Ws1_w": Ws1_w, "Ws1_b": Ws1_b, "Ws_w": Ws_w, "Ws_b": Ws_b,
    })
    res = run_bass_kernel_spmd(nc, in_maps, list(range(NCORES)))
    globals()["last_results"] = res
    out = np.concatenate([res.results[i]["out"] for i in range(NCORES)], axis=0)
    return out.astype(np.float32)
